# revision 54
# baseline (speedup 1.0000x reference)
"""Trainium2 Bass kernel for nn_HHGR (gnn_message_passing).

Strategy (8 NeuronCores, groups sharded 1024/core):
  - host: sum_tab = (user_table + user_embedding) in bf16, per-core shards
    of members/mask/H_gl^T (bf16), replicated small weights.
  - device per core:
      * gather member rows via indirect DMA (128 bf16 rows/call; the ~1.1us
        SWDGE descriptor-emission per call is the kernel's pacing floor)
      * attention: PE-transpose tiles -> hid -> per-tile K=16 logit matmuls
        straight into pcol columns -> one [128,4] exp (masked softmax without
        max-subtraction; logits are tiny so exp is safe)
      * weighted member sum via mask-block-diagonal matmuls -> g_att
      * X = g_att/denom + group_table; V = X @ hw1 (bf16)
      * AllGather V in 3 asymmetric pieces (sbs 0-3 mid-attention, 4-6 late,
        7 tiny at the end) so the collectives hide under the gather phase;
        Y^T = V^T H^T (K=8192 accumulation, H^T bf16 SBUF-resident, k-loop
        ordered by AG piece); relu; W = h @ hw2; AllGather W in halves with
        per-half SBUF loads; out^T = W^T H^T stored transposed
  - host: per-core transpose of out^T + concat shards.
"""
import sys
sys.path.insert(0, "/opt/trn_rl_repo")

import numpy as np
import ml_dtypes

import concourse.bass as bass
import concourse.bacc as bacc
import concourse.mybir as mybir
import concourse.tile as tile
from concourse.bass_utils import run_bass_kernel_spmd

F32 = mybir.dt.float32
BF16 = mybir.dt.bfloat16
I32 = mybir.dt.int32
AF = mybir.ActivationFunctionType

G, M, D, U = 8192, 32, 128, 200000
H_ATT = 16
NC = 8
GPC = G // NC          # 1024 groups per core
R = GPC * M            # 32768 rows per core
NT = R // 128          # 256 tiles of 128 rows
NB = NT // 4           # 64 batches of 4 tiles (512 rows)
NSB = GPC // 128       # 8 superblocks of 128 groups
BPS = NB // NSB        # 8 batches per superblock
KT = G // 128          # 64 K-tiles for the big matmuls
GB = 8                 # tiles per indirect-gather call

_CACHE = {}


def _build():
    nc = bacc.Bacc("TRN2", target_bir_lowering=False, debug=False)

    # ---- inputs ----
    sum_tab = nc.dram_tensor("sum_tab", [U, D], BF16, kind="ExternalInput")
    gidx = nc.dram_tensor("gidx", [128, NT], I32, kind="ExternalInput")
    mdiag = nc.dram_tensor("mdiag", [128, NT * 4], F32, kind="ExternalInput")
    gtab = nc.dram_tensor("gtab", [GPC, D], F32, kind="ExternalInput")
    ht = nc.dram_tensor("ht", [G, GPC], BF16, kind="ExternalInput")
    aw1 = nc.dram_tensor("aw1", [D, H_ATT], BF16, kind="ExternalInput")
    aw1x8 = nc.dram_tensor("aw1x8", [D, 128], BF16, kind="ExternalInput")
    aw2 = nc.dram_tensor("aw2", [H_ATT, 1], BF16, kind="ExternalInput")
    aw2rep = nc.dram_tensor("aw2rep", [128, 1], BF16, kind="ExternalInput")
    ab1rep = nc.dram_tensor("ab1rep", [128, 1], F32, kind="ExternalInput")
    ab2rep = nc.dram_tensor("ab2rep", [128, 1], F32, kind="ExternalInput")
    hw1 = nc.dram_tensor("hw1", [D, D], BF16, kind="ExternalInput")
    hw2 = nc.dram_tensor("hw2", [D, D], BF16, kind="ExternalInput")
    ab1c = nc.dram_tensor("ab1c", [H_ATT, 1], F32, kind="ExternalInput")
    ab2c = nc.dram_tensor("ab2c", [1, 1], F32, kind="ExternalInput")
    id_bf = nc.dram_tensor("id_bf", [128, 128], BF16, kind="ExternalInput")
    id_f32 = nc.dram_tensor("id_f32", [128, 128], F32, kind="ExternalInput")
    ones_col = nc.dram_tensor("ones_col", [128, 1], BF16, kind="ExternalInput")
    one11 = nc.dram_tensor("one11", [1, 1], F32, kind="ExternalInput")

    out = nc.dram_tensor("out", [D, GPC], F32, kind="ExternalOutput")

    # internal DRAM for collectives
    HGPC = GPC // 2
    dummy_in = nc.dram_tensor("dummy_in", [128, 1], BF16)
    dummy_out = nc.dram_tensor("dummy_out", [128 * NC, 1], BF16, addr_space="Shared")
    # V all-gathered in 3 asymmetric groups: sbs 0-3 (mid-attention),
    # sbs 4-6 (late attention), sb 7 (tiny, at the end)
    VSB = [(0, 4), (4, 3), (7, 1)]  # (first sb, n sbs)
    vdramg = [nc.dram_tensor(f"vdramg{i}", [n * 128, D], BF16)
              for i, (_, n) in enumerate(VSB)]
    vagg = [nc.dram_tensor(f"vagg{i}", [n * 128 * NC, D], BF16,
                           addr_space="Shared") for i, (_, n) in enumerate(VSB)]
    wdram1 = nc.dram_tensor("wdram1", [HGPC, D], BF16)
    wdram2 = nc.dram_tensor("wdram2", [HGPC, D], BF16)
    wag1 = nc.dram_tensor("wag1", [HGPC * NC, D], BF16, addr_space="Shared")
    wag2 = nc.dram_tensor("wag2", [HGPC * NC, D], BF16, addr_space="Shared")

    # group of local k-tile j, and tile slot of global k-tile kk within its
    # group's SBUF tile [128, n*NC*128]
    def vgrp(j):
        return 0 if j < 4 else (1 if j < 7 else 2)

    def vcol(kk):
        c, j = kk // 8, kk % 8
        g = vgrp(j)
        s0, n = VSB[g]
        return c * n + (j - s0)

    # W in halves: half h holds core c's local k-tiles [4h, 4h+4)
    def wcol(kk):
        c, j = kk // 8, kk % 8
        return c * 4 + (j % 4)

    with tile.TileContext(nc) as tc:
        with (
            tc.tile_pool(name="const", bufs=1) as cpool,
            tc.tile_pool(name="htpool", bufs=1) as htpool,
            tc.tile_pool(name="gath", bufs=12) as gpool,
            tc.tile_pool(name="work", bufs=3) as wpool,
            tc.tile_pool(name="sbx", bufs=1) as xpool,
            tc.tile_pool(name="ps_big", bufs=2, space="PSUM") as psb,
            tc.tile_pool(name="ps_small", bufs=1, space="PSUM") as pss,
            tc.tile_pool(name="ps_acc", bufs=2, space="PSUM") as psa,
        ):
            # ---- warm up the collectives path while attention runs ----
            nc.gpsimd.collective_compute(
                "AllGather",
                mybir.AluOpType.bypass,
                replica_groups=[list(range(NC))],
                ins=[dummy_in[:]],
                outs=[dummy_out[:]],
            )

            # ---- constants into SBUF (gather-feeding tables first) ----
            gidx_sb0 = cpool.tile([128, NT], I32, tag="gidx")
            nc.sync.dma_start(gidx_sb0[:], gidx[:])
            aw1_sb = cpool.tile([D, H_ATT], BF16, tag="aw1")
            nc.sync.dma_start(aw1_sb[:], aw1[:])
            aw1x8_sb = cpool.tile([D, 128], BF16, tag="aw1x8")
            nc.sync.dma_start(aw1x8_sb[:], aw1x8[:])
            aw2rep_sb = cpool.tile([128, 1], BF16, tag="aw2rep")
            nc.sync.dma_start(aw2rep_sb[:], aw2rep[:])
            ab1rep_sb = cpool.tile([128, 1], F32, tag="ab1rep")
            nc.sync.dma_start(ab1rep_sb[:], ab1rep[:])
            ab2rep_sb = cpool.tile([128, 1], F32, tag="ab2rep")
            nc.sync.dma_start(ab2rep_sb[:], ab2rep[:])
            aw2_sb = cpool.tile([H_ATT, 1], BF16, tag="aw2")
            nc.sync.dma_start(aw2_sb[:], aw2[:])
            hw1_sb = cpool.tile([D, D], BF16, tag="hw1")
            nc.sync.dma_start(hw1_sb[:], hw1[:])
            hw2_sb = cpool.tile([D, D], BF16, tag="hw2")
            nc.sync.dma_start(hw2_sb[:], hw2[:])
            ab1_sb = cpool.tile([H_ATT, 1], F32, tag="ab1")
            nc.sync.dma_start(ab1_sb[:], ab1c[:])
            ab2_sb = cpool.tile([1, 1], F32, tag="ab2")
            nc.sync.dma_start(ab2_sb[:], ab2c[:])
            idbf_sb = cpool.tile([128, 128], BF16, tag="idbf")
            nc.sync.dma_start(idbf_sb[:], id_bf[:])
            idf_sb = cpool.tile([128, 128], F32, tag="idf")
            nc.sync.dma_start(idf_sb[:], id_f32[:])
            ones_sb = cpool.tile([128, 1], BF16, tag="ones")
            nc.sync.dma_start(ones_sb[:], ones_col[:])
            one11_sb = cpool.tile([1, 1], F32, tag="one11")
            nc.sync.dma_start(one11_sb[:], one11[:])
            gidx_sb = gidx_sb0
            # mdiag feeds only the DVE mask build (first needed ~1.5us in);
            # loading it after gidx keeps the first gather off its DMA lane
            mdiag_sb = cpool.tile([128, NT * 4], F32, tag="mdiag")
            nc.sync.dma_start(mdiag_sb[:], mdiag[:])
            # prefetch all group-table rows up front (keeps DMA lanes free of
            # small mid-attention loads)
            gtab_all = cpool.tile([128, NSB * 128], F32, tag="gtab_all")
            nc.sync.dma_start(
                gtab_all[:].rearrange("p (s d) -> p s d", d=128),
                gtab.rearrange("(s p) d -> p s d", p=128),
            )

            # ---- H^T resident [128, KT*GPC] bf16 (16MB) ----
            ht_sb = htpool.tile([128, KT * GPC], BF16, tag="ht")
            for k in range(KT):
                nc.sync.dma_start(
                    ht_sb[:, k * GPC : (k + 1) * GPC],
                    ht[k * 128 : (k + 1) * 128, :],
                )

            # ---- attention over 8 superblocks ----
            xt_all = xpool.tile([128, GPC], BF16, tag="xt")  # X^T per core
            vag_t = [
                xpool.tile([128, n * NC * 128], BF16, tag=f"vag{i}",
                           name=f"vag_t{i}")
                for i, (_, n) in enumerate(VSB)
            ]
            wag_t = [
                xpool.tile([128, 4 * NC * 128], BF16, tag=f"wag{h}",
                           name=f"wag_t{h}")
                for h in range(2)
            ]
            for sb in range(NSB):
                g_attT = psa.tile([128, 128], F32, tag="gatt", space="PSUM")
                maskp_sb = wpool.tile([128, 128], BF16, tag="maskp")
                for b in range(BPS):
                    bb = sb * BPS + b
                    gbfs = []
                    embT_ps = psb.tile([128, 512], BF16, tag="bigbf", space="PSUM")
                    for j in range(4):
                        t = 4 * bb + j
                        gbf = gpool.tile([128, 128], BF16, tag="g")
                        nc.gpsimd.indirect_dma_start(
                            out=gbf[:],
                            out_offset=None,
                            in_=sum_tab[:],
                            in_offset=bass.IndirectOffsetOnAxis(
                                ap=gidx_sb[:, t : t + 1], axis=0
                            ),
                        )
                        nc.tensor.transpose(
                            embT_ps[:, j * 128 : (j + 1) * 128], gbf[:], idbf_sb[:]
                        )
                        gbfs.append(gbf)
                    embT_sb = wpool.tile([128, 512], BF16, tag="embT")
                    nc.vector.tensor_copy(embT_sb[:], embT_ps[:])
                    hidT_ps = pss.tile([H_ATT, 512], F32, tag="hidT", space="PSUM")
                    nc.tensor.matmul(
                        hidT_ps[:], aw1_sb[:], embT_sb[:], start=True, stop=True
                    )
                    hidT_sb = wpool.tile([H_ATT, 512], BF16, tag="hidT_sb")
                    nc.scalar.activation(
                        hidT_sb[:], hidT_ps[:], AF.Relu, bias=ab1_sb[:, :1]
                    )
                    # 4 K=16 logit matmuls -> pcol columns directly, then one
                    # [128,4] exp (replaces logit-row MM + [1,512] exp + 4 K=1
                    # transpose MMs + DVE copy)
                    pcol_ps = pss.tile([128, 4], F32, tag="hidT", space="PSUM")
                    for j in range(4):
                        nc.tensor.matmul(
                            pcol_ps[:, j : j + 1],
                            hidT_sb[:, j * 128 : (j + 1) * 128],
                            aw2_sb[:],
                            start=True,
                            stop=True,
                        )
                    pcol_sb = wpool.tile([128, 4], F32, tag="pcol_sb")
                    nc.scalar.activation(
                        pcol_sb[:], pcol_ps[:], AF.Exp, bias=ab2rep_sb[:, :1]
                    )
                    # maskp[:, (j, gl)] = pcol[:, j] * mdiag[:, (t, gl)]
                    nc.vector.tensor_tensor(
                        maskp_sb[:, b * 16 : (b + 1) * 16].rearrange(
                            "p (j l) -> p j l", j=4
                        ),
                        pcol_sb[:].rearrange("p (j o) -> p j o", j=4).to_broadcast(
                            [128, 4, 4]
                        ),
                        mdiag_sb[:, 16 * bb : 16 * (bb + 1)].rearrange(
                            "p (j l) -> p j l", j=4
                        ),
                        mybir.AluOpType.mult,
                    )
                    for j in range(4):
                        tloc = 4 * b + j
                        nc.tensor.matmul(
                            g_attT[:, 4 * tloc : 4 * tloc + 4],
                            gbfs[j][:],
                            maskp_sb[:, b * 16 + 4 * j : b * 16 + 4 * j + 4],
                            start=True,
                            stop=True,
                        )
                # ---- superblock tail ----
                den_ps = pss.tile([128, 4], F32, tag="hidT", space="PSUM")
                nc.tensor.matmul(
                    den_ps[:, 0:1], maskp_sb[:], ones_sb[:], start=True, stop=True
                )
                recip_sb = wpool.tile([128, 1], F32, tag="recip")
                nc.vector.reciprocal(recip_sb[:], den_ps[:, 0:1])
                # transpose g_attT -> natural [g, d]
                gat_sb = wpool.tile([128, 128], F32, tag="gat")
                nc.vector.tensor_copy(gat_sb[:], g_attT[:])
                gan_ps = psb.tile([128, 128], F32, tag="big", space="PSUM")
                nc.tensor.transpose(gan_ps[:], gat_sb[:], idf_sb[:])
                x_nat = wpool.tile([128, 128], BF16, tag="xnat")
                nc.vector.scalar_tensor_tensor(
                    x_nat[:],
                    gan_ps[:],
                    recip_sb[:, :1],
                    gtab_all[:, sb * 128 : (sb + 1) * 128],
                    mybir.AluOpType.mult,
                    mybir.AluOpType.add,
                )
                xt_ps = psb.tile([128, 128], BF16, tag="bigbf", space="PSUM")
                nc.tensor.transpose(xt_ps[:], x_nat[:], idbf_sb[:])
                nc.vector.tensor_copy(
                    xt_all[:, sb * 128 : (sb + 1) * 128], xt_ps[:]
                )
                # V = X @ hw1  (natural [g, d'])
                v_ps = psb.tile([128, 128], F32, tag="big", space="PSUM")
                nc.tensor.matmul(
                    v_ps[:],
                    xt_all[:, sb * 128 : (sb + 1) * 128],
                    hw1_sb[:],
                    start=True,
                    stop=True,
                )
                v_sb = wpool.tile([128, 128], BF16, tag="vsb")
                nc.vector.tensor_copy(v_sb[:], v_ps[:])
                g = vgrp(sb)
                s0, _n = VSB[g]
                nc.sync.dma_start(
                    vdramg[g][(sb - s0) * 128 : (sb - s0 + 1) * 128, :], v_sb[:]
                )
                if sb - s0 == _n - 1:  # last sb of its AG group
                    nc.gpsimd.collective_compute(
                        "AllGather",
                        mybir.AluOpType.bypass,
                        replica_groups=[list(range(NC))],
                        ins=[vdramg[g][:]],
                        outs=[vagg[g][:]],
                    )
                    # stage SBUF load right behind the AG -> overlaps attention
                    nc.sync.dma_start(
                        vag_t[g][:].rearrange("p (k d) -> p k d", d=128),
                        vagg[g].rearrange("(k p) d -> p k d", p=128),
                    )

            # ---- stage 1: Y^T = V^T H^T ; h = relu(Y) ----
            # k-loop ordered by AG group so PE starts while later AGs finish
            kk_order = [c * 8 + j for jg in (range(4), range(4, 7), range(7, 8))
                        for j in jg for c in range(8)]
            ht_all = xpool.tile([128, GPC], BF16, tag="hT")
            for c2 in range(2):
                y_ps = psb.tile([128, 512], F32, tag="big", space="PSUM")
                for i, kk in enumerate(kk_order):
                    col = vcol(kk)
                    nc.tensor.matmul(
                        y_ps[:],
                        vag_t[vgrp(kk % 8)][:, col * 128 : (col + 1) * 128],
                        ht_sb[:, kk * GPC + c2 * 512 : kk * GPC + c2 * 512 + 512],
                        start=(i == 0),
                        stop=(i == KT - 1),
                    )
                nc.scalar.activation(
                    ht_all[:, c2 * 512 : (c2 + 1) * 512], y_ps[:], AF.Relu
                )

                # ---- W = h @ hw2 (natural), half-AllGather ----
                for s2 in range(4):
                    sb2 = c2 * 4 + s2
                    w_ps = psb.tile([128, 128], F32, tag="big", space="PSUM")
                    nc.tensor.matmul(
                        w_ps[:],
                        ht_all[:, sb2 * 128 : (sb2 + 1) * 128],
                        hw2_sb[:],
                        start=True,
                        stop=True,
                    )
                    w_sb = wpool.tile([128, 128], BF16, tag="wsb")
                    nc.vector.tensor_copy(w_sb[:], w_ps[:])
                    wd = wdram1 if c2 == 0 else wdram2
                    nc.sync.dma_start(wd[s2 * 128 : (s2 + 1) * 128, :], w_sb[:])
                nc.gpsimd.collective_compute(
                    "AllGather",
                    mybir.AluOpType.bypass,
                    replica_groups=[list(range(NC))],
                    ins=[(wdram1 if c2 == 0 else wdram2)[:]],
                    outs=[(wag1 if c2 == 0 else wag2)[:]],
                )
                # load this W half right away; overlaps the other stage-1 chunk
                nc.sync.dma_start(
                    wag_t[c2][:].rearrange("p (k d) -> p k d", d=128),
                    (wag1 if c2 == 0 else wag2).rearrange("(k p) d -> p k d", p=128),
                )

            # ---- stage 2: out^T = W^T H^T (stored transposed; host untransposes)
            kk_order_w = [c * 8 + h * 4 + j for h in range(2) for c in range(8)
                          for j in range(4)]
            outT = xpool.tile([128, GPC], F32, tag="outT")
            for c2 in range(2):
                o_ps = psb.tile([128, 512], F32, tag="big", space="PSUM")
                for i, kk in enumerate(kk_order_w):
                    col = wcol(kk)
                    nc.tensor.matmul(
                        o_ps[:],
                        wag_t[0 if (kk % 8) < 4 else 1][:, col * 128 : (col + 1) * 128],
                        ht_sb[:, kk * GPC + c2 * 512 : kk * GPC + c2 * 512 + 512],
                        start=(i == 0),
                        stop=(i == KT - 1),
                    )
                nc.vector.tensor_copy(outT[:, c2 * 512 : (c2 + 1) * 512], o_ps[:])
                nc.sync.dma_start(
                    out[:, c2 * 512 : (c2 + 1) * 512],
                    outT[:, c2 * 512 : (c2 + 1) * 512],
                )

    nc.compile()
    return nc


def _prep_inputs(group_inputs, members, member_mask, user_embedding, H_gl,
                 user_table, group_table, aw1, ab1, aw2, ab2, hw1, hw2):
    bf = ml_dtypes.bfloat16
    sum_tab = (
        np.asarray(user_table, np.float32) + np.asarray(user_embedding, np.float32)
    ).astype(bf)
    Ht = np.asarray(H_gl, np.float32)
    gi = np.asarray(group_inputs, np.int64)
    gtab_full = np.asarray(group_table, np.float32)[gi]

    consts = dict(
        aw1=np.asarray(aw1, np.float32).astype(bf),
        aw2=np.asarray(aw2, np.float32).astype(bf),
        hw1=np.asarray(hw1, np.float32).astype(bf),
        hw2=np.asarray(hw2, np.float32).astype(bf),
        ab1c=np.asarray(ab1, np.float32).reshape(H_ATT, 1).copy(),
        ab2c=np.asarray(ab2, np.float32).reshape(1, 1).copy(),
        aw1x8=np.kron(
            np.ones((1, 4), np.float32),
            np.pad(np.asarray(aw1, np.float32), ((0, 0), (0, 32 - H_ATT))),
        ).astype(bf),
        aw2rep=np.kron(np.ones((8, 1), np.float32),
                       np.asarray(aw2, np.float32).reshape(H_ATT, 1)
                       ).astype(bf),
        ab1rep=np.kron(np.ones((8, 1), np.float32),
                       np.asarray(ab1, np.float32).reshape(H_ATT, 1)),
        ab2rep=np.full((128, 1), float(np.asarray(ab2).reshape(-1)[0]),
                       np.float32),
        id_bf=np.eye(128, dtype=np.float32).astype(bf),
        id_f32=np.eye(128, dtype=np.float32),
        ones_col=np.ones((128, 1), np.float32).astype(bf),
        one11=np.ones((1, 1), np.float32),
        sum_tab=sum_tab,
        dummy_in=np.zeros((128, 1), np.float32).astype(bf),
    )

    p = np.arange(128)
    gl_p = p // 32
    m_p = p % 32
    in_maps = []
    for c in range(NC):
        sl = slice(c * GPC, (c + 1) * GPC)
        mem = np.asarray(members, np.int64)[sl].astype(np.int32).reshape(-1)
        gidx = np.ascontiguousarray(mem.reshape(NT, 128).T)
        mask01 = (np.asarray(member_mask, np.float32)[sl] > 0).astype(np.float32)
        # val[p, t] = mask01[4t + p//32, p%32]
        t_idx = np.arange(NT)
        val = mask01[(4 * t_idx[None, :] + gl_p[:, None]), m_p[:, None]]  # [128, NT]
        mdiag = np.zeros((128, NT, 4), np.float32)
        mdiag[p, :, gl_p] = val
        mdiag = np.ascontiguousarray(mdiag.reshape(128, NT * 4))
        ht_c = np.ascontiguousarray(Ht[sl].T).astype(bf)
        in_maps.append(
            dict(
                consts,
                gidx=gidx,
                mdiag=mdiag,
                gtab=np.ascontiguousarray(gtab_full[sl]),
                ht=ht_c,
            )
        )
    return in_maps


def kernel(**inputs):
    if "nc" not in _CACHE:
        _CACHE["nc"] = _build()
    nc = _CACHE["nc"]
    in_maps = _prep_inputs(**inputs)
    res = run_bass_kernel_spmd(nc, in_maps, core_ids=list(range(NC)))
    out = np.concatenate(
        [np.ascontiguousarray(res.results[c]["out"].T) for c in range(NC)], axis=0
    )
    return out.astype(np.float32)


if __name__ == "__main__":
    import reference
    inp = {k: np.asarray(v) for k, v in reference.setup_inputs().items()}
    exp = np.asarray(reference.reference(**inp))
    got = kernel(**inp)
    err = np.abs(got - exp).max() / (np.abs(exp).max() + 1e-30)
    rel = np.linalg.norm(got - exp) / (np.linalg.norm(exp) + 1e-30)
    print(f"absmax-rel: {err:.3e}  fro-rel: {rel:.3e}")
